# revision 1
# baseline (speedup 1.0000x reference)
"""Trainium2 Bass kernel for nn_Attention_48661979463892.

Multi-head attention: B=2, H=8, dk=dv=64, T=S=2048, E=512.
  keys    = Wk @ x[b]          -> per head [64, T]
  values  = Wv @ x[b]          -> per head [64, T]
  queries = Wq @ y[b]          -> per head [64, S]
  scores  = keys^T @ queries + mask            [T, S]
  attn    = softmax(0.125 * scores, axis=T)    (normalize over keys axis)
  out     = values @ attn                      [64, S]
  res     = W @ concat_heads(out) + b          -> [B, S, O]

Sharding: 16 (batch, head) pairs over 8 cores -> core c handles batch c//4,
head-pair c%4 (global head rows 128*(c%4) .. +128).  Each core emits a
partial [S, O] contribution of the final linear (its 128 v-channels); the
host sums 4 partials per batch and adds the bias.

On-device layout per core:
  scores are computed tile-wise as [t_tile=128, s_chunk=512] blocks (both
  heads sharing one [128, 1024] PSUM pair) so softmax's reduce axis (t) is
  the PSUM accumulation axis of the AV matmul; the softmax denominator
  comes from a ones-column appended to values^T (M=65 AV matmul).  The
  1/colsum division is deferred past the per-head final linear: tiny K=1
  matmuls transpose each [1, 128] colsum slice into a PSUM column, one
  [128, 8] reciprocal inverts them, and the per-partition scales are fused
  into the PSUM->SBUF drain of the final-linear results (tensor_scalar +
  scalar_tensor_tensor).  The whole epilogue of chunk sc is software-
  pipelined into the t-loop of chunk sc+1, and dummy warm-up matmuls keep
  the PE's HAM clock-gate hot while the input DMAs land.
"""

import numpy as np

N_CORES = 8
B, I, T, S, O = 2, 512, 2048, 2048, 512
H_PER_CORE = 2
DK = 64
SCALING = DK ** -0.5  # 0.125

# matmul input dtype for the bulk matmuls:
#   "f32"  exact, 4 cyc/col, self-loading weights
#   "f32r" bf16-pair split, 1 cyc/col at N>=512, but self-loading weights
#          (LDWEIGHTS cannot overlap -> ~2x slower in practice)
#   "bf16" 1 cyc/col, pipelined LDWEIGHTS + FWL
MM_DTYPE = "bf16"
N_WARMUP_MM = 14

_BUILD_CACHE = {}


def _split_multi_waits(nc):
    """walrus in this toolchain accepts only ONE sync wait per instruction.
    Hoist extra waits onto same-engine NoOps inserted just before."""
    import concourse.mybir as mybir

    ctr = 0
    for fn in nc.m.functions:
        for blk in fn.blocks:
            new_insts = []
            for inst in blk.instructions:
                si = inst.sync_info
                if si is not None and len(si.on_wait) > 1:
                    waits = list(si.on_wait)
                    for w in waits[:-1]:
                        ctr += 1
                        nop = mybir.InstNoOp(
                            name=f"waitsplit-{ctr}", ins=[], outs=[]
                        )
                        nop.engine = inst.engine
                        nop.sync_info = mybir.SyncInfo(on_wait=[w], on_update=[])
                        new_insts.append(nop)
                    del si.on_wait[:-1]
                new_insts.append(inst)
            blk.instructions[:] = new_insts


def _build(with_mask):
    import concourse.bass as bass
    import concourse.mybir as mybir
    import concourse.tile as tile
    from concourse.bass import ts, ds

    f32 = mybir.dt.float32
    mmdt = {
        "f32": f32,
        "f32r": mybir.dt.float32r,
        "bf16": mybir.dt.bfloat16,
    }[MM_DTYPE]
    nc = bass.Bass()
    x_p = nc.declare_dram_parameter("x4", [128, 4, T], mmdt, isOutput=False)
    y_p = nc.declare_dram_parameter("y4", [4, 128, 4, 512], mmdt, isOutput=False)
    wk_p = nc.declare_dram_parameter("wkT", [128, 4, 128], mmdt, isOutput=False)
    wv_p = nc.declare_dram_parameter("wvT", [128, 4, 128], mmdt, isOutput=False)
    wq_p = nc.declare_dram_parameter("wqT", [128, 4, 128], mmdt, isOutput=False)
    wc_p = nc.declare_dram_parameter("wcT", [2, 64, O], mmdt, isOutput=False)
    if with_mask:
        mask_p = nc.declare_dram_parameter("maskT", [16, 128, S], f32, isOutput=False)
    res_p = nc.declare_dram_parameter("res", [S, O], f32, isOutput=True)

    N_SC = S // 512    # s chunks of 512
    N_TT = T // 128    # t tiles of 128

    with tile.TileContext(nc) as tc:
        with (
            nc.allow_low_precision(reason="float32r/bf16 matmul operands"),
            tc.tile_pool(name="consts", bufs=1) as consts,
            tc.tile_pool(name="exps", bufs=4) as exps_pool,
            tc.tile_pool(name="epi", bufs=2) as epi_pool,
            tc.tile_pool(name="osb", bufs=4) as osb_pool,
            tc.tile_pool(name="osc", bufs=4) as osc_pool,
            tc.tile_pool(name="resout", bufs=3) as res_pool,
            tc.tile_pool(name="ps_scores", bufs=2, space="PSUM") as ps_scores_pool,
            tc.tile_pool(name="ps_acc", bufs=2, space="PSUM") as ps_acc_pool,
            tc.tile_pool(name="ps_misc", bufs=2, space="PSUM") as ps_misc_pool,
        ):
            # dummy matmuls on scratch data keep the PE busy while the input
            # DMAs land, so the HAM clock-gate is warm when real work starts
            scratch_sb = consts.tile([128, 512], mmdt)
            nc.vector.memset(scratch_sb, 0.0)
            for w in range(N_WARMUP_MM):
                ps_w = ps_scores_pool.tile([128, 1024], f32, tag="ps_s", name="ps_w")
                nc.tensor.matmul(
                    ps_w[:, 0:512], scratch_sb[:, 0:128], scratch_sb,
                    start=True, stop=True,
                )

            # ---------------- load inputs ----------------
            wk_sb = consts.tile([128, 4, 128], mmdt)
            wv_sb = consts.tile([128, 4, 128], mmdt)
            wq_sb = consts.tile([128, 4, 128], mmdt)
            wc_sb0 = consts.tile([64, O], mmdt)
            wc_sb1 = consts.tile([64, O], mmdt)
            x_sb = consts.tile([128, 4, T], mmdt)
            y_sb = consts.tile([128, 4, S], mmdt)
            # one strided DMA per tensor; x + the weights the first matmuls
            # need go first, y arrives in s-chunk slices (queries n0 first)
            nc.gpsimd.dma_start(out=wk_sb, in_=wk_p[:, :, :])
            nc.gpsimd.dma_start(out=wq_sb, in_=wq_p[:, :, :])
            nc.sync.dma_start(out=x_sb[:, 0:2, :], in_=x_p[:, 0:2, :])
            nc.scalar.dma_start(out=x_sb[:, 2:3, :], in_=x_p[:, 2:3, :])
            nc.gpsimd.dma_start(out=x_sb[:, 3:4, :], in_=x_p[:, 3:4, :])
            nc.scalar.dma_start(out=y_sb[:, :, 0:512], in_=y_p[0])
            nc.gpsimd.dma_start(out=wv_sb, in_=wv_p[:, :, :])
            nc.gpsimd.dma_start(out=wc_sb0, in_=wc_p[0])
            nc.gpsimd.dma_start(out=wc_sb1, in_=wc_p[1])
            for n in range(1, S // 512):
                eng = nc.sync if n % 2 == 0 else nc.scalar
                eng.dma_start(out=y_sb[:, :, ts(n, 512)], in_=y_p[n])

            # ---------------- projections ----------------
            keys_sb = consts.tile([128, T], mmdt)
            qs_sb = consts.tile([128, S], mmdt)

            def project2(dst, w_sb, src, n0, fillers=0):
                """project n-slices n0 and n0+1 with the j loop outermost so
                each contraction chunk is consumed as its DMA lands.
                `fillers` adds scratch matmuls between j groups to keep the
                HAM clock-gate warm while waiting for the next chunk."""
                ps0 = ps_misc_pool.tile([128, 512], f32, tag="misc", name="ps0")
                ps1 = ps_misc_pool.tile([128, 512], f32, tag="misc", name="ps1")
                for j in range(4):
                    for ps, n in ((ps0, n0), (ps1, n0 + 1)):
                        nc.tensor.matmul(
                            ps,
                            w_sb[:, j, :],
                            src[:, j, ts(n, 512)],
                            start=(j == 0),
                            stop=(j == 3),
                        )
                    if j < 3:
                        for w in range(fillers):
                            ps_w = ps_scores_pool.tile(
                                [128, 1024], f32, tag="ps_s", name="ps_w"
                            )
                            nc.tensor.matmul(
                                ps_w[:, 0:512], scratch_sb[:, 0:128], scratch_sb,
                                start=True, stop=True,
                            )
                nc.vector.tensor_copy(out=dst[:, ts(n0, 512)], in_=ps0)
                nc.vector.tensor_copy(out=dst[:, ts(n0 + 1, 512)], in_=ps1)

            project2(keys_sb, wk_sb, x_sb, 0, fillers=6)

            # values^T with ones columns: [t_part=128, tt, 130]
            # cols 0:64 head0, col 64 ones, cols 65:129 head1, col 129 ones.
            # The per-tile projection matmuls are emitted inline in the first
            # t_loop (just-in-time before each tile's AV) so the main loop
            # starts as soon as keys + the first queries slice exist.
            valT_sb = consts.tile([128, N_TT, 130], mmdt)
            nc.vector.memset(valT_sb[:, :, 64:65], 1.0)
            nc.vector.memset(valT_sb[:, :, 129:130], 1.0)

            def valT_proj(tt):
                ps = ps_misc_pool.tile([128, 128], f32, tag="misc", name="ps")
                for j in range(4):
                    nc.tensor.matmul(
                        ps,
                        x_sb[:, j, ts(tt, 128)],
                        wv_sb[:, j, :],
                        start=(j == 0),
                        stop=(j == 3),
                    )
                nc.vector.tensor_copy(out=valT_sb[:, tt, 0:64], in_=ps[:, 0:64])
                nc.vector.tensor_copy(out=valT_sb[:, tt, 65:129], in_=ps[:, 64:128])

            project2(qs_sb, wq_sb, y_sb, 0)
            project2(keys_sb, wk_sb, x_sb, 2)
            project2(qs_sb, wq_sb, y_sb, 2)

            def extra_work(tt):
                if tt == 0:
                    valT_proj(0); valT_proj(1)
                elif tt + 1 < N_TT:
                    valT_proj(tt + 1)

            # ---------------- attention main loop (software-pipelined) ----
            def t_loop(sc, prev_osb):
                """scores + exp + AV accumulation for s chunk `sc`; the
                previous chunk's normalize is issued after the first tile so
                its DVE latency hides inside this chunk's PE/ACT stream.
                Returns the two [65, 512] unnormalized AV+colsum results,
                copied to SBUF (f32r) so the PSUM banks free up early."""
                osc_prev = None
                ps_o = [
                    ps_acc_pool.tile([65, 512], f32, tag="av", name=f"ps_o{h}")
                    for h in range(2)
                ]
                for tt in range(N_TT):
                    if sc == 0:
                        extra_work(tt)
                    if tt == 1 and prev_osb is not None:
                        osc_prev = normalize(prev_osb)
                    ps_s = ps_scores_pool.tile([128, 1024], f32, tag="ps_s", name="ps_s")
                    if with_mask:
                        m_sb = exps_pool.tile([128, 512], f32, tag="mask", name="m_sb")
                        nc.sync.dma_start(out=m_sb, in_=mask_p[tt][:, ts(sc, 512)])
                    for h in range(2):
                        nc.tensor.matmul(
                            ps_s[:, ts(h, 512)],
                            keys_sb[64 * h : 64 * h + 64, ts(tt, 128)],
                            qs_sb[64 * h : 64 * h + 64, ts(sc, 512)],
                            start=True,
                            stop=True,
                        )
                        if with_mask:
                            nc.vector.tensor_tensor(
                                ps_s[:, ts(h, 512)],
                                ps_s[:, ts(h, 512)],
                                m_sb,
                                mybir.AluOpType.add,
                            )
                    ex = exps_pool.tile([128, 1024], mmdt)
                    nc.scalar.activation(
                        out=ex,
                        in_=ps_s,
                        func=mybir.ActivationFunctionType.Exp,
                        scale=float(SCALING),
                    )
                    for h in range(2):
                        nc.tensor.matmul(
                            ps_o[h],
                            valT_sb[:, tt, 65 * h : 65 * h + 65],
                            ex[:, ts(h, 512)],
                            start=(tt == 0),
                            stop=(tt == N_TT - 1),
                        )
                osb = []
                for h in range(2):
                    o_un = osb_pool.tile([65, 512], mmdt, tag=f"osb{h}", name=f"osb{h}")
                    if sc == N_SC - 1 and h == 1:
                        nc.scalar.copy(o_un, ps_o[h])
                    else:
                        nc.vector.tensor_copy(out=o_un, in_=ps_o[h])
                    osb.append(o_un)
                return osb, osc_prev

            def normalize(osb):
                """1/colsum as per-partition columns: transpose each [1,128]
                colsum slice into a PSUM column via a K=1 matmul, then one
                tiny [128, 8] reciprocal.  Column h*4+st holds head h,
                s-subtile st."""
                cs_ps = ps_misc_pool.tile([128, 8], f32, tag="misc", name="cs_ps")
                one_mm = valT_sb[64:65, 0, 64:65]
                for h in range(2):
                    for st in range(4):
                        nc.tensor.matmul(
                            cs_ps[:, h * 4 + st : h * 4 + st + 1],
                            osb[h][64:65, ts(st, 128)],
                            one_mm,
                            start=True,
                            stop=True,
                        )
                rec_col = epi_pool.tile([128, 8], f32, tag="rec", name="rec_col")
                nc.vector.reciprocal(out=rec_col, in_=cs_ps)
                return rec_col

            def epilogue(sc, osb, rec_col):
                """per-head final linear, then per-partition 1/colsum scaling
                fused into the PSUM->SBUF drain; store.  For the last chunk
                the scores PSUM pool is dead, so fin pairs borrow its banks
                (deeper pipelining) and the idle ACT takes half the scales."""
                last = sc == N_SC - 1
                for st in range(4):
                    if last and st < 2:
                        pr = ps_scores_pool.tile(
                            [128, 1024], f32, tag="ps_s", name="ps_rp"
                        )
                        ps_r0 = pr[:, 0:512]
                        ps_r1 = pr[:, 512:1024]
                    else:
                        ps_r0 = ps_misc_pool.tile(
                            [128, 512], f32, tag="misc", name="ps_r0"
                        )
                        ps_r1 = ps_misc_pool.tile(
                            [128, 512], f32, tag="misc", name="ps_r1"
                        )
                    nc.tensor.matmul(
                        ps_r0, osb[0][0:64, ts(st, 128)], wc_sb0,
                        start=True, stop=True,
                    )
                    nc.tensor.matmul(
                        ps_r1, osb[1][0:64, ts(st, 128)], wc_sb1,
                        start=True, stop=True,
                    )
                    a_sb = res_pool.tile([128, O], f32, tag="a_sb", name="a_sb")
                    if last:
                        nc.scalar.activation(
                            out=a_sb,
                            in_=ps_r0,
                            func=mybir.ActivationFunctionType.Copy,
                            scale=rec_col[:, st : st + 1],
                        )
                    else:
                        nc.vector.tensor_scalar_mul(
                            a_sb, ps_r0, rec_col[:, st : st + 1]
                        )
                    r_sb = res_pool.tile([128, O], f32)
                    nc.vector.scalar_tensor_tensor(
                        out=r_sb,
                        in0=ps_r1,
                        scalar=rec_col[:, 4 + st : 5 + st],
                        in1=a_sb,
                        op0=mybir.AluOpType.mult,
                        op1=mybir.AluOpType.add,
                    )
                    nc.sync.dma_start(
                        out=res_p[ds(sc * 512 + st * 128, 128), :], in_=r_sb
                    )

            prev_osb = None
            for sc in range(N_SC):
                osb, rec_prev = t_loop(sc, prev_osb)
                if rec_prev is not None:
                    epilogue(sc - 1, prev_osb, rec_prev)
                prev_osb = osb
            epilogue(N_SC - 1, prev_osb, normalize(prev_osb))

    _split_multi_waits(nc)
    return nc


def _get_nc(with_mask):
    key = (with_mask, MM_DTYPE)
    if key not in _BUILD_CACHE:
        _BUILD_CACHE[key] = _build(with_mask)
    return _BUILD_CACHE[key]


def _mm_np_dtype():
    if MM_DTYPE == "bf16":
        import ml_dtypes
        return np.dtype(ml_dtypes.bfloat16)
    return np.dtype(np.float32)


def _make_in_maps(x, y, mask, Wk, Wv, Wq, W, with_mask):
    mdt = _mm_np_dtype()
    in_maps = []
    for c in range(N_CORES):
        bb, hp = divmod(c, 4)
        e_sl = slice(128 * hp, 128 * hp + 128)
        im = {
            "x4": np.ascontiguousarray(
                x[bb].reshape(4, 128, T).transpose(1, 0, 2).astype(mdt)
            ),
            "y4": np.ascontiguousarray(
                y[bb].reshape(4, 128, 4, 512).transpose(2, 1, 0, 3).astype(mdt)
            ),
            "wkT": np.ascontiguousarray(
                Wk[e_sl].T.reshape(4, 128, 128).transpose(1, 0, 2).astype(mdt)
            ),
            "wvT": np.ascontiguousarray(
                Wv[e_sl].T.reshape(4, 128, 128).transpose(1, 0, 2).astype(mdt)
            ),
            "wqT": np.ascontiguousarray(
                Wq[e_sl].T.reshape(4, 128, 128).transpose(1, 0, 2).astype(mdt)
            ),
            "wcT": np.ascontiguousarray(
                np.stack(
                    [
                        W[:, 128 * hp : 128 * hp + 64].T,
                        W[:, 128 * hp + 64 : 128 * hp + 128].T,
                    ]
                ).astype(mdt)
            ),
        }
        if with_mask:
            im["maskT"] = np.ascontiguousarray(mask.reshape(16, 128, S))
        in_maps.append(im)
    return in_maps


def kernel(x, y, mask, Wk, Wv, Wq, W, b):
    from concourse.bass_utils import run_bass_kernel_spmd

    x = np.asarray(x, dtype=np.float32)
    y = np.asarray(y, dtype=np.float32)
    mask = np.asarray(mask, dtype=np.float32)
    Wk = np.asarray(Wk, dtype=np.float32)
    Wv = np.asarray(Wv, dtype=np.float32)
    Wq = np.asarray(Wq, dtype=np.float32)
    W = np.asarray(W, dtype=np.float32)
    b = np.asarray(b, dtype=np.float32)

    with_mask = bool(np.any(mask))
    nc = _get_nc(with_mask)
    in_maps = _make_in_maps(x, y, mask, Wk, Wv, Wq, W, with_mask)

    r = run_bass_kernel_spmd(nc, in_maps, core_ids=list(range(N_CORES)))
    parts = [r.results[c]["res"] for c in range(N_CORES)]
    out = np.stack(
        [
            parts[0] + parts[1] + parts[2] + parts[3],
            parts[4] + parts[5] + parts[6] + parts[7],
        ],
        axis=0,
    )
    out += b[None, None, :]
    return out.astype(np.float32)

